# revision 5
# baseline (speedup 1.0000x reference)
"""Trainium2 Bass kernel for nn_Net_57956288692302 (GNN message passing).

Strategy (8 NeuronCores, SPMD):
  * Key observation: the reference's per-edge MLP (h2v/sigmoid) only feeds the
    last column of the edge message, and the aggregated last column is
    overwritten by ``aggr.at[:, -1].set(x[:, -1])`` — so each conv layer is
    LINEAR in x given the graph:
        x' = relu([A0 @ (x@w0) + A1 @ (x@w1)](cols 0:64) | x[:,64]) + x@root + bias)
  * Nodes are range-sharded across 8 cores (6250 nodes each; the var/con split
    at node 25000 lands exactly on the core-3/4 boundary). Edges are sharded by
    dst (dst-sorted), so every core aggregates only its own node range.
  * Per layer: every core projects its x-range through w0/w1 into a table
    chunk, an AllGather replicates the full [2*8*6400, 64] projected table,
    then each core does a per-edge dma_gather (256B rows) from the table and
    scatter-adds via one-hot matmuls on the TensorEngine (edges sorted by dst;
    one-hot built on the VectorEngine by iota-compare).
  * Node-level MLPs (input embed + final 4-layer head) run feature-major on
    the TensorEngine, data-parallel over nodes.
"""

import numpy as np

# ---------------- problem constants (hardcoded per contract) ----------------
NCORES = 8
N_VAR = 25000
N_CON = 25000
NN = 50000
DIM = 64
D1 = 65
RNG = 6250            # nodes per core
NGRP = 50             # 128-node groups in padded range
RPAD = NGRP * 128     # 6400
NCELLS = 49           # ceil(RNG / 128) scatter cells (128 nodes each)
CELLW = 128
NCHUNK = 4            # gather table chunks (int16 index reach)
CHUNK = 2 * RPAD * 2  # 25600 rows per chunk (2 ranks x 2 rel x 6400)
NG = NCHUNK * NCELLS  # (chunk, cell) scatter groups
NLAYER = 4
MAXBT = 16            # tiles per gather batch
NYG = 49              # 128-node groups used for table build (6272 cols)

TRACE = False         # test.py sets True to capture NTFF profile
LAST_EXEC_NS = None


# ---------------------------- host preprocessing ----------------------------
def _prep(inputs):
    vf = np.asarray(inputs["var_node_features"], np.float32)
    cf = np.asarray(inputs["con_node_features"], np.float32)
    ei = np.asarray(inputs["edge_index"], np.int64)
    et = np.asarray(inputs["edge_types"], np.int64)
    av = np.asarray(inputs["assoc_var"], np.int64)
    ac = np.asarray(inputs["assoc_con"], np.int64)
    p = inputs["params"]

    f = np.zeros(NN, np.float32)
    f[av] = vf[:, 0]
    f[ac] = cf[:, 0]

    src, dst = ei[0], ei[1]
    srank = src // RNG
    grow = srank * (2 * RPAD) + et * RPAD + (src - srank * RNG)
    chunk = grow // CHUNK
    crow = grow % CHUNK
    core = dst // RNG
    cell = (dst % RNG) // CELLW
    dloc = (dst % RNG) % CELLW
    g = chunk * NCELLS + cell  # group id in tile order (chunk-major)

    cnt = np.bincount(core * NG + g, minlength=NCORES * NG).reshape(NCORES, NG)
    gt = -(-cnt.max(0) // 128)  # tiles per group (shared across cores)
    tile_base = np.zeros(NG + 1, np.int64)
    tile_base[1:] = np.cumsum(gt)
    NT = int(tile_base[-1])

    tiles = []  # (cell, chunk, first, last) per tile
    for gg in range(NG):
        c, cl = gg // NCELLS, gg % NCELLS
        n = int(gt[gg])
        for i in range(n):
            tiles.append((cl, c, i == 0, i == n - 1))

    batches = []  # (tile_start, ntiles, chunk)
    cb = tile_base[np.arange(0, NG + 1, NCELLS)]
    for c in range(NCHUNK):
        t = int(cb[c])
        while t < int(cb[c + 1]):
            nt = min(MAXBT, int(cb[c + 1]) - t)
            batches.append((t, nt, c))
            t += nt

    gidx_slot = np.zeros((NCORES, NT * 128), np.int64)
    dloc_slot = np.full((NCORES, NT * 128), -1.0, np.float32)
    for r in range(NCORES):
        m = core == r
        gr, cr, dr = g[m], crow[m], dloc[m]
        o = np.argsort(gr, kind="stable")
        gr_s = gr[o]
        grp_start = np.searchsorted(gr_s, np.arange(NG))
        pos = np.arange(len(gr_s)) - grp_start[gr_s]
        slot = tile_base[gr_s] * 128 + pos
        gidx_slot[r, slot] = cr[o]
        dloc_slot[r, slot] = dr[o]

    gw = np.zeros((NCORES, 16, NT * 8), np.int16)
    for ts, nt, _c in batches:
        seq = gidx_slot[:, ts * 128:(ts + nt) * 128]  # [NCORES, nt*128]
        gw[:, :, ts * 8:(ts + nt) * 8] = seq.reshape(NCORES, -1, 16).transpose(0, 2, 1)
    gw = np.tile(gw, (1, 8, 1))  # replicate across the 8 Q7 stripes -> [NCORES,128,NT*8]
    dl = np.ascontiguousarray(
        dloc_slot.reshape(NCORES, NT, 128).transpose(0, 2, 1)
    )  # [NCORES, 128, NT]

    # conv params
    w01s, roots, biasb = [], [], []
    for c in p["convs"]:
        att = np.asarray(c["att"], np.float32)
        basis = np.asarray(c["basis"], np.float32)
        w = np.einsum("rb,bio->rio", att, basis)  # [2, 65, 64]
        w01s.append(np.concatenate([w[0], w[1]], axis=1))  # [65, 128]
        roots.append(np.asarray(c["root"], np.float32))
        biasb.append(np.tile(np.asarray(c["bias"], np.float32)[None, :], (128, 1)))
    w01s = np.stack(w01s)
    roots = np.stack(roots)
    biasb = np.stack(biasb)

    (vw1, vb1), (vw2, vb2) = p["var_mlp"]
    (cw1, cb1), (cw2, cb2) = p["con_mlp"]

    fpad = np.zeros((NCORES, RPAD), np.float32)
    fpad[:, :RNG] = f.reshape(NCORES, RNG)
    featT = fpad[:, None, :]  # [NCORES, 1, RPAD]
    featnm = np.ascontiguousarray(
        fpad.reshape(NCORES, NGRP, 128).transpose(0, 2, 1)
    )  # [NCORES, 128, NGRP]

    iota = np.tile(np.arange(128, dtype=np.float32), (128, MAXBT, 1))

    def a32(x):
        return np.ascontiguousarray(np.asarray(x, np.float32))

    shared = {
        "iota": iota,
        "w01s": a32(w01s),
        "roots": a32(roots),
        "biasb": a32(biasb),
        "fc1w": a32(np.asarray(p["fc1"][0], np.float32).reshape(5, D1, DIM).transpose(1, 0, 2)),
        "fc1b": a32(np.asarray(p["fc1"][1])[:, None]),
        "fc2w": a32(p["fc2"][0]),
        "fc2b": a32(np.asarray(p["fc2"][1])[:, None]),
        "fc3w": a32(p["fc3"][0]),
        "fc3b": a32(np.asarray(p["fc3"][1])[:, None]),
        "fc4w": a32(p["fc4"][0]),
        "fc4b": a32(np.asarray(p["fc4"][1]).reshape(1, 1)),
    }
    in_maps = []
    for r in range(NCORES):
        m = dict(shared)
        m["gw"] = np.ascontiguousarray(gw[r])
        m["dl"] = a32(dl[r])
        m["featT"] = a32(featT[r])
        m["featnm"] = a32(featnm[r])
        if r < 4:
            m["xw1"], m["xb1"] = a32(vw1), a32(np.asarray(vb1)[:, None])
            m["xw2"], m["xb2"] = a32(vw2), a32(np.asarray(vb2)[:, None])
        else:
            m["xw1"], m["xb1"] = a32(cw1), a32(np.asarray(cb1)[:, None])
            m["xw2"], m["xb2"] = a32(cw2), a32(np.asarray(cb2)[:, None])
        in_maps.append(m)
    return in_maps, NT, tiles, batches, av


# ------------------------------- device build -------------------------------
def _build(NT, tiles, batches):
    import concourse.mybir as mybir
    import concourse.tile as tile
    from concourse import bacc
    from concourse.masks import make_identity

    f32 = mybir.dt.float32
    i16 = mybir.dt.int16
    AG_ROWS = NCORES * 2 * RPAD

    nc = bacc.Bacc("TRN2", target_bir_lowering=False, debug=False,
                   num_devices=NCORES)

    def inp(name, shape, dt=f32):
        return nc.dram_tensor(name, shape, dt, kind="ExternalInput")

    gw_d = inp("gw", [128, NT * 8], i16)
    dl_d = inp("dl", [128, NT])
    iota_d = inp("iota", [128, MAXBT, 128])
    featT_d = inp("featT", [1, RPAD])
    featnm_d = inp("featnm", [128, NGRP])
    xw1_d = inp("xw1", [1, DIM])
    xb1_d = inp("xb1", [DIM, 1])
    xw2_d = inp("xw2", [DIM, DIM])
    xb2_d = inp("xb2", [DIM, 1])
    w01s_d = inp("w01s", [NLAYER, D1, 2 * DIM])
    roots_d = inp("roots", [NLAYER, D1, D1])
    biasb_d = inp("biasb", [NLAYER, 128, D1])
    fc_d = {}
    fc_d["fc1w"] = inp("fc1w", [D1, 5, DIM])
    fc_d["fc1b"] = inp("fc1b", [DIM, 1])
    for i in (2, 3):
        fc_d[f"fc{i}w"] = inp(f"fc{i}w", [DIM, DIM])
        fc_d[f"fc{i}b"] = inp(f"fc{i}b", [DIM, 1])
    fc_d["fc4w"] = inp("fc4w", [DIM, 1])
    fc_d["fc4b"] = inp("fc4b", [1, 1])
    out_d = nc.dram_tensor("out", [1, RPAD], f32, kind="ExternalOutput")

    RELU = mybir.ActivationFunctionType.Relu

    with tile.TileContext(nc) as tc:
        with (
            tc.tile_pool(name="const", bufs=1) as cp,
            tc.tile_pool(name="state", bufs=1) as sp,
            tc.tile_pool(name="dram", bufs=1, space="DRAM") as dp,
            tc.tile_pool(name="work", bufs=3) as wp,
            tc.tile_pool(name="gbp", bufs=3) as gbp,
            tc.tile_pool(name="selp", bufs=2) as selp,
        ):
            # ---- constant loads ----
            gw = cp.tile([128, NT * 8], i16)
            nc.sync.dma_start(gw[:], gw_d[:])
            dl = cp.tile([128, NT], f32)
            nc.sync.dma_start(dl[:], dl_d[:])
            iota = cp.tile([128, MAXBT, 128], f32)
            nc.sync.dma_start(iota[:], iota_d[:])
            xw1 = cp.tile([1, DIM], f32)
            nc.sync.dma_start(xw1[:], xw1_d[:])
            xb1 = cp.tile([DIM, 1], f32)
            nc.sync.dma_start(xb1[:], xb1_d[:])
            xw2 = cp.tile([DIM, DIM], f32)
            nc.sync.dma_start(xw2[:], xw2_d[:])
            xb2 = cp.tile([DIM, 1], f32)
            nc.sync.dma_start(xb2[:], xb2_d[:])
            w01 = [cp.tile([D1, 2 * DIM], f32, name=f"w01_{l}") for l in range(NLAYER)]
            root = [cp.tile([D1, D1], f32, name=f"root_{l}") for l in range(NLAYER)]
            bb = [cp.tile([128, D1], f32, name=f"bb_{l}") for l in range(NLAYER)]
            for l in range(NLAYER):
                nc.sync.dma_start(w01[l][:], w01s_d[l])
                nc.sync.dma_start(root[l][:], roots_d[l])
                nc.sync.dma_start(bb[l][:], biasb_d[l])
            fc = {}
            for k, d in fc_d.items():
                fc[k] = cp.tile(list(d.shape), f32, name=k)
                nc.sync.dma_start(fc[k][:], d[:])
            ident = cp.tile([128, 128], f32)
            make_identity(nc, ident[:])

            # ---- persistent state (2 ping-pong x buffers; history in DRAM) ----
            xT = [sp.tile([D1, RPAD], f32, name=f"xT{i}") for i in range(2)]
            x64 = [sp.tile([128, NGRP], f32, name=f"x64_{i}") for i in range(2)]
            aggr = sp.tile([128, NCELLS, D1], f32)
            nc.sync.dma_start(x64[0][:], featnm_d[:])

            AGin = dp.tile([2, RPAD, DIM], f32)
            YAGs = [dp.tile([AG_ROWS, DIM], f32, addr_space="Shared", name=f"YAG{l}")
                    for l in range(NLAYER)]
            xhist = dp.tile([NLAYER + 1, D1, RPAD], f32)

            # zero the table pad rows once (cols 6272:6400 of each relation)
            zt = cp.tile([128, DIM], f32)
            nc.vector.memset(zt[:], 0.0)
            for rel in range(2):
                nc.sync.dma_start(AGin[rel, NYG * 128:RPAD, :], zt[:])

            # ---- input MLP (feature-major), fills xT[0] ----
            blks = [(i * 512, 512) for i in range(12)] + [(6144, 256)]
            with tc.tile_pool(name="psum_in", bufs=2, space="PSUM") as pin:
                for b0, bn in blks:
                    sl = slice(b0, b0 + bn)
                    fT = wp.tile([1, 512], f32, tag="fT")
                    nc.sync.dma_start(fT[:, :bn], featT_d[:, sl])
                    h1p = pin.tile([DIM, 512], f32, tag="h1p")
                    nc.tensor.matmul(h1p[:, :bn], lhsT=xw1[:], rhs=fT[:, :bn],
                                     start=True, stop=True)
                    h1 = wp.tile([DIM, 512], f32, tag="h1")
                    nc.scalar.activation(h1[:, :bn], h1p[:, :bn], RELU,
                                         bias=xb1[:, :1])
                    h2p = pin.tile([DIM, 512], f32, tag="h2p")
                    nc.tensor.matmul(h2p[:, :bn], lhsT=xw2[:], rhs=h1[:, :bn],
                                     start=True, stop=True)
                    nc.vector.tensor_scalar_add(xT[0][0:DIM, sl], h2p[:, :bn],
                                                xb2[:, :1])
                    nc.vector.tensor_copy(xT[0][DIM:D1, sl], fT[:, :bn])
            nc.sync.dma_start(xhist[0], xT[0][:])

            # ---- conv layers ----
            with (
                tc.tile_pool(name="psum_mm", bufs=3, space="PSUM") as pmm,
                tc.tile_pool(name="psum_gp", bufs=4, space="PSUM") as pgp,
            ):
                for l in range(NLAYER):
                    YAG = YAGs[l]
                    xs, xd = xT[l % 2], xT[(l + 1) % 2]
                    x64c, x64n = x64[l % 2], x64[(l + 1) % 2]
                    # table build + AG input
                    for gi in range(NYG):
                        sl = slice(gi * 128, (gi + 1) * 128)
                        yp = pmm.tile([128, 2 * DIM], f32, tag="yp")
                        nc.tensor.matmul(yp[:], lhsT=xs[:, sl], rhs=w01[l][:],
                                         start=True, stop=True)
                        yt = wp.tile([128, 2 * DIM], f32, tag="yt")
                        nc.vector.tensor_copy(yt[:], yp[:])
                        nc.sync.dma_start(AGin[0, sl, :], yt[:, 0:DIM])
                        nc.sync.dma_start(AGin[1, sl, :], yt[:, DIM:2 * DIM])
                    nc.gpsimd.collective_compute(
                        "AllGather",
                        mybir.AluOpType.bypass,
                        replica_groups=[list(range(NCORES))],
                        ins=[AGin.opt()],
                        outs=[YAG.opt()],
                    )
                    # root term + aggr init (bias + x64 into col 64)
                    for cell in range(NCELLS):
                        sl = slice(cell * 128, (cell + 1) * 128)
                        rp = pmm.tile([128, D1], f32, tag="yp", name=f"rp{l}_{cell}")
                        nc.tensor.matmul(rp[:], lhsT=xs[:, sl], rhs=root[l][:],
                                         start=True, stop=True)
                        nc.vector.tensor_tensor(out=aggr[:, cell, :], in0=rp[:],
                                                in1=bb[l][:],
                                                op=mybir.AluOpType.add)
                        nc.vector.tensor_add(aggr[:, cell, DIM:D1],
                                             aggr[:, cell, DIM:D1],
                                             x64c[:, cell:cell + 1])
                    # gather + scatter
                    gp_tiles = {}
                    for ts, nt, c in batches:
                        gb = gbp.tile([128, MAXBT, DIM], f32, tag="gb")
                        nc.gpsimd.dma_gather(
                            out_ap=gb[:, 0:nt, :],
                            in_ap=YAG[c * CHUNK:(c + 1) * CHUNK, :],
                            idxs_ap=gw[:, ts * 8:(ts + nt) * 8],
                            num_idxs=nt * 128,
                            num_idxs_reg=nt * 128,
                            elem_size=DIM,
                            single_packet=False,
                        )
                        sel = selp.tile([128, MAXBT, CELLW], f32, tag="sel")
                        nc.vector.tensor_tensor(
                            out=sel[:, 0:nt, :],
                            in0=dl[:, ts:ts + nt].to_broadcast([128, nt, CELLW]),
                            in1=iota[:, 0:nt, :],
                            op=mybir.AluOpType.is_equal,
                        )
                        for i in range(nt):
                            cell, c2, first, last = tiles[ts + i]
                            key = (cell, c2)
                            if first:
                                gp_tiles[key] = pgp.tile([CELLW, DIM], f32, tag="gp",
                                                         name=f"gp{l}_{cell}_{c2}")
                            nc.tensor.matmul(gp_tiles[key][:], lhsT=sel[:, i, :],
                                             rhs=gb[:, i, :], start=first, stop=last)
                            if last:
                                gp = gp_tiles.pop(key)
                                nc.vector.tensor_add(aggr[:, cell, 0:DIM],
                                                     aggr[:, cell, 0:DIM], gp[:])
                    # epilogue: relu, save col64, transpose into next x
                    for cell in range(NCELLS):
                        nxm = wp.tile([128, D1], f32, tag="nxm")
                        nc.scalar.activation(nxm[:], aggr[:, cell, :], RELU)
                        nc.vector.tensor_copy(x64n[:, cell:cell + 1], nxm[:, DIM:D1])
                        tp = pmm.tile([D1, 128], f32, tag="yp", name=f"tp{l}_{cell}")
                        nc.tensor.transpose(out=tp[:], in_=nxm[:], identity=ident[:])
                        nc.vector.tensor_copy(xd[:, cell * 128:(cell + 1) * 128], tp[:])
                    nc.sync.dma_start(xhist[l + 1], xd[:])

            # ---- final MLP (feature-major over own nodes; x history from DRAM) ----
            fblks = [(i * 512, 512) for i in range(12)] + [(6144, 128)]
            with (
                tc.tile_pool(name="psum_fc", bufs=3, space="PSUM") as pfc,
                tc.tile_pool(name="xlp", bufs=7) as xlp,
            ):
                for b0, bn in fblks:
                    sl = slice(b0, b0 + bn)
                    h1p = pfc.tile([DIM, 512], f32, tag="hp")
                    for l in range(NLAYER + 1):
                        xl = xlp.tile([D1, 512], f32, tag="xl")
                        nc.sync.dma_start(xl[:, :bn], xhist[l, :, sl])
                        nc.tensor.matmul(h1p[:, :bn],
                                         lhsT=fc["fc1w"][:, l, :],
                                         rhs=xl[:, :bn],
                                         start=(l == 0), stop=(l == NLAYER))
                    h = wp.tile([DIM, 512], f32, tag="h1")
                    nc.scalar.activation(h[:, :bn], h1p[:, :bn], RELU,
                                         bias=fc["fc1b"][:, :1])
                    for i in (2, 3):
                        hp = pfc.tile([DIM, 512], f32, tag="hp", name=f"h{i}p_{b0}")
                        nc.tensor.matmul(hp[:, :bn], lhsT=fc[f"fc{i}w"][:],
                                         rhs=h[:, :bn], start=True, stop=True)
                        h = wp.tile([DIM, 512], f32, tag="h1", name=f"h{i}_{b0}")
                        nc.scalar.activation(h[:, :bn], hp[:, :bn], RELU,
                                             bias=fc[f"fc{i}b"][:, :1])
                    h4p = pfc.tile([1, 512], f32, tag="h4p")
                    nc.tensor.matmul(h4p[:, :bn], lhsT=fc["fc4w"][:], rhs=h[:, :bn],
                                     start=True, stop=True)
                    ob = wp.tile([1, 512], f32, tag="ob")
                    nc.vector.tensor_scalar_add(ob[:, :bn], h4p[:, :bn],
                                                fc["fc4b"][:, :1])
                    nc.sync.dma_start(out_d[:, sl], ob[:, :bn])

    nc.compile()
    return nc


# --------------------------------- entry ------------------------------------
def kernel(**inputs):
    global LAST_EXEC_NS
    from concourse.bass_utils import run_bass_kernel_spmd

    in_maps, NT, tiles, batches, av = _prep(inputs)
    nc = _build(NT, tiles, batches)

    kwargs = {}
    if TRACE:
        import sys
        import types

        import trn_agent_boot.trn_boot as tb
        import concourse.bass_utils as bu

        hook = tb._ntff_profile_via_ctypes("/opt/axon/libaxon_pjrt.so")
        mod = types.ModuleType("antenv.axon_hooks")
        mod.get_axon_ntff_profile_hook = lambda: hook
        mod.set_axon_ntff_profile_hook = lambda h: None
        sys.modules["antenv.axon_hooks"] = mod
        bu.upload_artifacts = lambda tmpdir: tmpdir
        kwargs["trace"] = True

    res = run_bass_kernel_spmd(nc, in_maps, core_ids=list(range(NCORES)), **kwargs)
    LAST_EXEC_NS = res.exec_time_ns

    out_full = np.concatenate([res.results[r]["out"][0, :RNG] for r in range(NCORES)])
    return out_full[av].astype(np.float32)
